# revision 17
# baseline (speedup 1.0000x reference)
"""AFT-Full distributed Trainium2 kernel (v2).

Reference computation (B=8, T=4096, D=512, H=64):
    Q = x @ wq.T ; K = x @ wk.T ; V = x @ wv.T                     [B,T,H]
    ew  = exp(wbias)                                               [T,T]
    num = ew @ (exp(K)*V) ; den = ew @ exp(K)                      [B,T,H]
    out = (sigmoid(Q) * num/den) @ wp.T + bp                       [B,T,D]

Sharding: 4 batch-groups x 2 t-groups (core c: batches {2*(c//2),
2*(c//2)+1}, t-slice c%2).  No collectives.

v2 key ideas vs the 147us baseline:
- ew = 1 + r decomposition: host ships r = SCALE*(exp(wbias)-1) in
  fp8e3m4 (half the HBM bytes of bf16 wbias, no on-chip exp at all);
  the phase-2 matmul takes the fp8 tile directly as the moving operand
  (mixed bf16 x fp8 matmul, 1 cyc/row).  The missing "1 @ Z" rank-1
  term is a per-(batch,h) column sum of Z, accumulated on PE via tiny
  [.,1] matmuls with z-chunks as weights, then added in phase 3 as a
  per-partition Identity-activation bias.  SCALE cancels in num/den.
- Host-packed DMA layouts: x and r arrive via few large DMAs with 8KB
  contiguous rows, interleaved on one queue in exact consumption order.
- Phase 2 is split into 4 t-passes of 512 cols; each pass's phase-3
  epilogue + projection + store is spliced between the next passes'
  matmul groups, hiding the tail and keeping PE continuously busy
  (TRN2 PE clock ramps only after ~3us of uninterrupted work).
"""

import sys

for _p in ("/opt/trn_rl_repo", "/opt/pypackages"):
    if _p not in sys.path:
        sys.path.append(_p)

import numpy as np
import ml_dtypes

B, T, D, H = 8, 4096, 512, 64
BG, TG = 4, 2            # batch groups x t groups = 8 cores
BPC = B // BG            # batches per core
TPC = T // TG            # t rows per core
NG = 4                   # x s-groups of 1024
NRG = 8                  # r s-groups of 512 (4 chunks)
NP = 4                   # phase-2/3 t-passes of 512
TP = TPC // NP
SCALE = 64.0
N_CORES = 8

_NC_CACHE = {}


def _build_module(use_bias):
    import concourse.bass as bass
    import concourse.mybir as mybir
    import concourse.tile as tile
    from concourse import bacc
    from contextlib import ExitStack

    bf16 = mybir.dt.bfloat16
    f32 = mybir.dt.float32
    f8 = mybir.dt.float8e3
    Exp = mybir.ActivationFunctionType.Exp
    Sigmoid = mybir.ActivationFunctionType.Sigmoid
    Identity = mybir.ActivationFunctionType.Identity
    mult = mybir.AluOpType.mult

    nc = bacc.Bacc("TRN2", target_bir_lowering=False, debug=False,
                   num_devices=N_CORES)

    xp = nc.dram_tensor("xp", [BPC, NG, 128, 4096], bf16,
                        kind="ExternalInput").ap()
    rp = nc.dram_tensor("rp", [NRG, 128, 4 * TPC], f8,
                        kind="ExternalInput").ap()
    wkv = nc.dram_tensor("wkv", [D, 2 * H], bf16, kind="ExternalInput").ap()
    wqT = nc.dram_tensor("wqT", [D, H], bf16, kind="ExternalInput").ap()
    wpT = nc.dram_tensor("wpT", [H + 1, D], bf16, kind="ExternalInput").ap()
    bkv = nc.dram_tensor("bkv", [1, 2 * H], bf16, kind="ExternalInput").ap()
    bqv = nc.dram_tensor("bqv", [H, 1], f32, kind="ExternalInput").ap()
    ones = nc.dram_tensor("ones", [1, 128], bf16, kind="ExternalInput").ap()
    out = nc.dram_tensor("out", [BPC, TPC, D], bf16,
                         kind="ExternalOutput").ap()

    with tile.TileContext(nc) as tc, ExitStack() as ctx:
        wpool = ctx.enter_context(tc.tile_pool(name="wts", bufs=1))
        xpool = ctx.enter_context(tc.tile_pool(name="xg", bufs=2 * NG))
        rpool = ctx.enter_context(tc.tile_pool(name="rr", bufs=NRG))
        zpool = ctx.enter_context(tc.tile_pool(name="z", bufs=BPC))
        sqpool = ctx.enter_context(tc.tile_pool(name="sq", bufs=BPC))
        cpool = ctx.enter_context(tc.tile_pool(name="csv", bufs=1))
        ytpool = ctx.enter_context(tc.tile_pool(name="yt", bufs=3))
        tpool = ctx.enter_context(tc.tile_pool(name="tmp", bufs=3))
        opool = ctx.enter_context(tc.tile_pool(name="osb", bufs=4))
        pps = ctx.enter_context(tc.tile_pool(name="pps", bufs=2,
                                             space="PSUM"))
        pnd = ctx.enter_context(tc.tile_pool(name="pnd", bufs=4,
                                             space="PSUM"))
        pcs = ctx.enter_context(tc.tile_pool(name="pcs", bufs=2,
                                             space="PSUM"))

        # --- resident weights / constants (sync queue, ahead of x) ---
        wkv_sb = wpool.tile([128, 4 * 2 * H], bf16)     # [128, 512]
        for d in range(4):
            nc.sync.dma_start(wkv_sb[:, d * 2 * H:(d + 1) * 2 * H],
                              wkv[d * 128:(d + 1) * 128, :])
        wq_sb = wpool.tile([128, 4 * H], bf16)          # [128, 256]
        for d in range(4):
            nc.sync.dma_start(wq_sb[:, d * H:(d + 1) * H],
                              wqT[d * 128:(d + 1) * 128, :])
        wp_sb = wpool.tile([H + 1, D], bf16)            # [65, 512]
        nc.sync.dma_start(wp_sb[:, :], wpT[:, :])
        bkv_sb = wpool.tile([1, 2 * H], bf16)
        nc.sync.dma_start(bkv_sb[:, :], bkv[:, :])
        bq_sb = wpool.tile([H, 1], f32)
        nc.sync.dma_start(bq_sb[:, :], bqv[:, :])
        ones_sb = wpool.tile([1, 128], bf16)
        nc.sync.dma_start(ones_sb[:, :], ones[:, :])
        col64 = wpool.tile([128, 1], bf16, name="col64")
        nc.gpsimd.memset(col64[:, :], SCALE)
        one1 = wpool.tile([1, 1], f32, name="one1")
        nc.gpsimd.memset(one1[:, :], 1.0)

        # --- input streams spread over 3 trigger queues, emission in
        # consumption order so arrival roughly tracks the compute sweep ---
        xg = [[None] * NG for _ in range(BPC)]
        rr = [None] * NRG
        _q = [0]
        _engs = (nc.sync, nc.gpsimd, nc.scalar)

        def _eng():
            e = _engs[_q[0] % 3]
            _q[0] += 1
            return e

        def x_dma(g):
            for b in range(BPC):
                t_ = xpool.tile([128, 4096], bf16, name=f"x{b}g{g}",
                                tag="xg")
                _eng().dma_start(t_[:, :], xp[b, g])
                xg[b][g] = t_

        def r_dma(rg):
            t_ = rpool.tile([128, 4 * TPC], f8, name=f"r{rg}", tag="rr")
            _eng().dma_start(t_[:, :], rp[rg])
            rr[rg] = t_

        x_dma(0)
        x_dma(1)
        r_dma(0)
        x_dma(2)
        r_dma(1)
        x_dma(3)
        for rg in range(2, NRG):
            r_dma(rg)

        z_sb = [zpool.tile([128, 32 * 128], bf16, name=f"z{b}", tag="z")
                for b in range(BPC)]
        sq_sb = [sqpool.tile([H, TPC], f32, name=f"sq{b}", tag="sq")
                 for b in range(BPC)]
        # one full PSUM bank per batch: start_tensor_calc zeroes a 2KB
        # region, so interleaved accumulation groups must not share a bank
        cs_ps = [pcs.tile([128, 512], f32, name=f"cs{b}", tag="cs")
                 for b in range(BPC)]

        # --- phase 1: Z = [eKV | eK] per 512-s block; colsum via tiny
        # z-as-weights matmuls accumulating [2H,1] per batch ---
        def ph1_group(g):
            for b in range(BPC):
                for sg in range(2):
                    pkv = pps.tile([128, 512], f32, name="pkv", tag="ps")
                    for si in range(4):
                        sc = g * 8 + sg * 4 + si
                        xoff = (sg * 4 + si) * 128
                        for d in range(4):
                            nc.tensor.matmul(
                                pkv[:, si * 128:(si + 1) * 128],
                                lhsT=xg[b][g][:, d * 1024 + xoff:
                                              d * 1024 + xoff + 128],
                                rhs=wkv_sb[:, d * 128:(d + 1) * 128],
                                start=(d == 0),
                                stop=(not use_bias and d == 3))
                        if use_bias:
                            nc.tensor.matmul(
                                pkv[:, si * 128:(si + 1) * 128],
                                lhsT=ones_sb[:, :], rhs=bkv_sb[:, :],
                                start=False, stop=True)
                    c0 = (g * 8 + sg * 4) * 128
                    pk3 = pkv[:, :].rearrange("p (c k) -> p c k", c=4)
                    zg3 = z_sb[b][:, c0:c0 + 512].rearrange(
                        "p (c k) -> p c k", c=4)
                    nc.scalar.activation(
                        zg3[:, :, H:2 * H], pk3[:, :, 0:H], Exp)
                    nc.vector.tensor_tensor(
                        zg3[:, :, 0:H], pk3[:, :, H:2 * H],
                        zg3[:, :, H:2 * H], mult)
                for si in range(8):
                    sc = g * 8 + si
                    nc.tensor.matmul(
                        cs_ps[b][:, 0:1],
                        lhsT=z_sb[b][:, sc * 128:(sc + 1) * 128],
                        rhs=col64[:, :],
                        start=(sc == 0), stop=(sc == 31),
                        skip_group_check=True)

        def q_block(b):
            for qb in range(4):
                pq = pps.tile([128, 512], f32, name="pq", tag="ps")
                for d in range(4):
                    nc.tensor.matmul(
                        pq[0:H, :],
                        lhsT=wq_sb[:, d * H:(d + 1) * H],
                        rhs=xg[b][qb // 2][:, d * 1024 + (qb % 2) * 512:
                                           d * 1024 + (qb % 2) * 512 + 512],
                        start=(d == 0), stop=(d == 3))
                nc.scalar.activation(
                    sq_sb[b][:, qb * 512:(qb + 1) * 512], pq[0:H, :],
                    Sigmoid, bias=bq_sb[:, :])

        # --- phase 2: nd[p][b] += z_sc^T @ r_sc over all 32 s-chunks ---
        nd = [[None] * BPC for _ in range(NP)]

        def nd_alloc(passes):
            for p in passes:
                for b in range(BPC):
                    nd[p][b] = pnd.tile([128, TP], f32, name=f"nd{p}{b}",
                                        tag="nd")

        def ph2_rg(rg, passes):
            for j in range(4):
                sc = rg * 4 + j
                for b in range(BPC):
                    for p in passes:
                        nc.tensor.matmul(
                            nd[p][b][:, :],
                            lhsT=z_sb[b][:, sc * 128:(sc + 1) * 128],
                            rhs=rr[rg][:, j * TPC + p * TP:
                                       j * TPC + (p + 1) * TP],
                            start=(sc == 0), stop=(sc == 31))

        # --- phase 3 ---
        csv = cpool.tile([128, BPC], f32, name="csv", tag="csv")
        tmps = {}

        def ph3_reads(p, b):
            ncp = tpool.tile([H, TP], f32, name="ncp", tag="ncp")
            nc.scalar.activation(ncp[:, :], nd[p][b][0:H, :], Identity,
                                 bias=csv[0:H, b:b + 1])
            dcp = tpool.tile([H, TP], f32, name="dcp", tag="dcp")
            nc.scalar.activation(dcp[:, :], nd[p][b][H:2 * H, :], Identity,
                                 bias=csv[H:2 * H, b:b + 1])
            rec = tpool.tile([H, TP], f32, name="rec", tag="rec")
            nc.vector.reciprocal_approx_fast(rec[:, :], dcp[:, :])
            tmp = tpool.tile([H, TP], f32, name="tmp", tag="tmp")
            nc.vector.tensor_tensor(tmp[:, :], ncp[:, :], rec[:, :], mult)
            tmps[(p, b)] = tmp

        def ph3_tail(p, b):
            yt = ytpool.tile([H + 1, TP], bf16, name="yt", tag="yt")
            nc.gpsimd.memset(yt[H:H + 1, :], 1.0)
            nc.gpsimd.tensor_tensor(
                yt[0:H, :], tmps[(p, b)][:, :],
                sq_sb[b][:, p * TP:(p + 1) * TP], mult)
            for c in range(4):
                po = pps.tile([128, 512], f32, name="po", tag="ps")
                nc.tensor.matmul(po[:, :], lhsT=yt[:, c * 128:(c + 1) * 128],
                                 rhs=wp_sb[:, :], start=True, stop=True)
                osb = opool.tile([128, 512], bf16, name="osb", tag="osb")
                if c % 2 == 0:
                    nc.scalar.copy(osb[:, :], po[:, :])
                else:
                    nc.vector.tensor_copy(osb[:, :], po[:, :])
                t0 = p * TP + c * 128
                eng = (nc.gpsimd, nc.sync)[c % 2]
                eng.dma_start(out[b, t0:t0 + 128, :], osb[:, :])

        # --- emission schedule (PE order = program order per engine) ---
        ph1_group(0)
        ph1_group(1)
        q_block(0)
        q_block(1)
        ph1_group(2)
        nd_alloc((0, 1))
        ph2_rg(0, (0, 1))
        ph2_rg(1, (0, 1))
        ph1_group(3)
        for b in range(BPC):
            nc.scalar.copy(csv[:, b:b + 1], cs_ps[b][:, 0:1])
        for rg in range(2, NRG):
            ph2_rg(rg, (0, 1))
        for p in (0, 1):
            for b in range(BPC):
                ph3_reads(p, b)
        nd_alloc((2,))
        ph2_rg(0, (2,))
        ph2_rg(1, (2,))
        ph3_tail(0, 0)
        ph3_tail(0, 1)
        ph2_rg(2, (2,))
        ph2_rg(3, (2,))
        ph3_tail(1, 0)
        ph3_tail(1, 1)
        for rg in range(4, NRG):
            ph2_rg(rg, (2,))
        for b in range(BPC):
            ph3_reads(2, b)
        nd_alloc((3,))
        ph2_rg(0, (3,))
        ph2_rg(1, (3,))
        ph3_tail(2, 0)
        ph2_rg(2, (3,))
        ph2_rg(3, (3,))
        ph3_tail(2, 1)
        for rg in range(4, NRG):
            ph2_rg(rg, (3,))
        for b in range(BPC):
            ph3_reads(3, b)
        ph3_tail(3, 0)
        ph3_tail(3, 1)

    nc.compile()
    from concourse.bass_interp import get_hw_module
    nc.m = get_hw_module(nc.m)
    return nc


def _get_module(use_bias):
    key = ("nc", use_bias)
    if key not in _NC_CACHE:
        _NC_CACHE[key] = _build_module(use_bias)
    return _NC_CACHE[key]


def kernel(x, wq, bq, wk, bk, wv, bv, wp, bp, wbias):
    from concourse.bass_utils import run_bass_kernel_spmd

    bf16 = ml_dtypes.bfloat16
    f8 = ml_dtypes.float8_e3m4
    x = np.asarray(x, np.float32)
    wbias = np.asarray(wbias, np.float32)
    wq, wk, wv, wp = (np.asarray(a, np.float32) for a in (wq, wk, wv, wp))
    bq, bk, bv, bp = (np.asarray(a, np.float32) for a in (bq, bk, bv, bp))

    xT_full = np.ascontiguousarray(x.transpose(0, 2, 1)).astype(bf16)
    rq_full = (SCALE * (np.exp(wbias) - 1.0)).T     # [s, t] f32

    wkv_h = np.concatenate([wk.T, wv.T], axis=1).astype(bf16)      # [D, 2H]
    wqT_h = np.ascontiguousarray(wq.T).astype(bf16)                # [D, H]
    wpT_h = np.concatenate(
        [wp.T, np.asarray(bp, np.float32)[None, :]], axis=0).astype(bf16)
    bkv_h = np.concatenate([bk, bv])[None, :].astype(bf16)         # [1, 2H]
    bq_h = np.asarray(bq, np.float32)[:, None].copy()              # [H, 1]
    ones_h = np.ones((1, 128), dtype=bf16)
    use_bias = bool(np.any(bk) or np.any(bv))

    # Per t-group: s-permuted inputs (own t-slice rows first) so the SPMD
    # graph reads Q's x columns at [0:TPC] on every core.
    perm = {}
    for tj in range(TG):
        perm[tj] = np.concatenate([
            np.arange(tj * TPC, (tj + 1) * TPC),
            np.arange(0, tj * TPC),
            np.arange((tj + 1) * TPC, T)])

    rp_tj = {}
    for tj in range(TG):
        rq = rq_full[perm[tj]][:, tj * TPC:(tj + 1) * TPC].astype(f8)
        rp_tj[tj] = np.ascontiguousarray(
            rq.reshape(NRG, 4, 128, TPC).transpose(0, 2, 1, 3)
            .reshape(NRG, 128, 4 * TPC))

    xp_c = {}
    for bi in range(BG):
        for tj in range(TG):
            xt = xT_full[bi * BPC:(bi + 1) * BPC][:, :, perm[tj]]
            xp_c[(bi, tj)] = np.ascontiguousarray(
                xt.reshape(BPC, 4, 128, NG, 1024).transpose(0, 3, 2, 1, 4)
                .reshape(BPC, NG, 128, 4096))

    in_maps = []
    for c in range(N_CORES):
        bi, tj = c // TG, c % TG
        in_maps.append({
            "xp": xp_c[(bi, tj)],
            "rp": rp_tj[tj],
            "wkv": wkv_h, "wqT": wqT_h, "wpT": wpT_h,
            "bkv": bkv_h, "bqv": bq_h, "ones": ones_h,
        })

    nc = _get_module(use_bias)
    res = run_bass_kernel_spmd(nc, in_maps, core_ids=list(range(N_CORES)))

    full = np.empty((B, T, D), dtype=np.float32)
    for c in range(N_CORES):
        bi, tj = c // TG, c % TG
        full[bi * BPC:(bi + 1) * BPC, tj * TPC:(tj + 1) * TPC, :] = \
            res.results[c]["out"].astype(np.float32)
    return full


# revision 18
# speedup vs baseline: 1.0513x; 1.0513x over previous
"""AFT-Full distributed Trainium2 kernel (v2).

Reference computation (B=8, T=4096, D=512, H=64):
    Q = x @ wq.T ; K = x @ wk.T ; V = x @ wv.T                     [B,T,H]
    ew  = exp(wbias)                                               [T,T]
    num = ew @ (exp(K)*V) ; den = ew @ exp(K)                      [B,T,H]
    out = (sigmoid(Q) * num/den) @ wp.T + bp                       [B,T,D]

Sharding: 4 batch-groups x 2 t-groups (core c: batches {2*(c//2),
2*(c//2)+1}, t-slice c%2).  No collectives.

v2 key ideas vs the 147us baseline:
- ew = 1 + r decomposition: host ships r = SCALE*(exp(wbias)-1) in
  fp8e3m4 (half the HBM bytes of bf16 wbias, no on-chip exp at all);
  the phase-2 matmul takes the fp8 tile directly as the moving operand
  (mixed bf16 x fp8 matmul, 1 cyc/row).  The missing "1 @ Z" rank-1
  term is a per-(batch,h) column sum of Z, accumulated on PE via tiny
  [.,1] matmuls with z-chunks as weights, then added in phase 3 as a
  per-partition Identity-activation bias.  SCALE cancels in num/den.
- Host-packed DMA layouts: x and r arrive via few large DMAs with 8KB
  contiguous rows, interleaved on one queue in exact consumption order.
- Phase 2 is split into 4 t-passes of 512 cols; each pass's phase-3
  epilogue + projection + store is spliced between the next passes'
  matmul groups, hiding the tail and keeping PE continuously busy
  (TRN2 PE clock ramps only after ~3us of uninterrupted work).
"""

import sys

for _p in ("/opt/trn_rl_repo", "/opt/pypackages"):
    if _p not in sys.path:
        sys.path.append(_p)

import numpy as np
import ml_dtypes

B, T, D, H = 8, 4096, 512, 64
BG, TG = 4, 2            # batch groups x t groups = 8 cores
BPC = B // BG            # batches per core
TPC = T // TG            # t rows per core
NG = 4                   # x s-groups of 1024
NRG = 8                  # r s-groups of 512 (4 chunks)
NP = 4                   # phase-2/3 t-passes of 512
TP = TPC // NP
SCALE = 64.0
N_CORES = 8

_NC_CACHE = {}


def _build_module(use_bias):
    import concourse.bass as bass
    import concourse.mybir as mybir
    import concourse.tile as tile
    from concourse import bacc
    from contextlib import ExitStack

    bf16 = mybir.dt.bfloat16
    f32 = mybir.dt.float32
    f8 = mybir.dt.float8e3
    Exp = mybir.ActivationFunctionType.Exp
    Sigmoid = mybir.ActivationFunctionType.Sigmoid
    Identity = mybir.ActivationFunctionType.Identity
    mult = mybir.AluOpType.mult

    nc = bacc.Bacc("TRN2", target_bir_lowering=False, debug=False,
                   num_devices=N_CORES)

    xp = nc.dram_tensor("xp", [BPC, NG, 128, 4096], bf16,
                        kind="ExternalInput").ap()
    rp = nc.dram_tensor("rp", [NRG, 128, 4 * TPC], f8,
                        kind="ExternalInput").ap()
    wkv = nc.dram_tensor("wkv", [D, 2 * H], bf16, kind="ExternalInput").ap()
    wqT = nc.dram_tensor("wqT", [D, H], bf16, kind="ExternalInput").ap()
    wpT = nc.dram_tensor("wpT", [H + 1, D], bf16, kind="ExternalInput").ap()
    bkv = nc.dram_tensor("bkv", [1, 2 * H], bf16, kind="ExternalInput").ap()
    bqv = nc.dram_tensor("bqv", [H, 1], f32, kind="ExternalInput").ap()
    ones = nc.dram_tensor("ones", [1, 128], bf16, kind="ExternalInput").ap()
    out = nc.dram_tensor("out", [BPC, TPC, D], bf16,
                         kind="ExternalOutput").ap()

    with tile.TileContext(nc) as tc, ExitStack() as ctx:
        wpool = ctx.enter_context(tc.tile_pool(name="wts", bufs=1))
        xpool = ctx.enter_context(tc.tile_pool(name="xg", bufs=2 * NG))
        rpool = ctx.enter_context(tc.tile_pool(name="rr", bufs=NRG))
        zpool = ctx.enter_context(tc.tile_pool(name="z", bufs=BPC))
        sqpool = ctx.enter_context(tc.tile_pool(name="sq", bufs=BPC))
        cpool = ctx.enter_context(tc.tile_pool(name="csv", bufs=1))
        ytpool = ctx.enter_context(tc.tile_pool(name="yt", bufs=3))
        tpool = ctx.enter_context(tc.tile_pool(name="tmp", bufs=3))
        opool = ctx.enter_context(tc.tile_pool(name="osb", bufs=4))
        pps = ctx.enter_context(tc.tile_pool(name="pps", bufs=2,
                                             space="PSUM"))
        pnd = ctx.enter_context(tc.tile_pool(name="pnd", bufs=4,
                                             space="PSUM"))
        pcs = ctx.enter_context(tc.tile_pool(name="pcs", bufs=2,
                                             space="PSUM"))

        # --- resident weights / constants (sync queue, ahead of x) ---
        wkv_sb = wpool.tile([128, 4 * 2 * H], bf16)     # [128, 512]
        for d in range(4):
            nc.sync.dma_start(wkv_sb[:, d * 2 * H:(d + 1) * 2 * H],
                              wkv[d * 128:(d + 1) * 128, :])
        wq_sb = wpool.tile([128, 4 * H], bf16)          # [128, 256]
        for d in range(4):
            nc.sync.dma_start(wq_sb[:, d * H:(d + 1) * H],
                              wqT[d * 128:(d + 1) * 128, :])
        wp_sb = wpool.tile([H + 1, D], bf16)            # [65, 512]
        nc.sync.dma_start(wp_sb[:, :], wpT[:, :])
        bkv_sb = wpool.tile([1, 2 * H], bf16)
        nc.sync.dma_start(bkv_sb[:, :], bkv[:, :])
        bq_sb = wpool.tile([H, 1], f32)
        nc.sync.dma_start(bq_sb[:, :], bqv[:, :])
        ones_sb = wpool.tile([1, 128], bf16)
        nc.sync.dma_start(ones_sb[:, :], ones[:, :])
        col64 = wpool.tile([128, 1], bf16, name="col64")
        nc.gpsimd.memset(col64[:, :], SCALE)
        one1 = wpool.tile([1, 1], f32, name="one1")
        nc.gpsimd.memset(one1[:, :], 1.0)

        # --- input streams: x in consumption order on sync; r on gpsimd,
        # held back behind the first two x groups so phase 1 gets full
        # bandwidth at the start ---
        from concourse.tile import add_dep_helper
        xg = [[None] * NG for _ in range(BPC)]
        rr = [None] * NRG
        x_dmas = []

        def x_dma(g):
            for b in range(BPC):
                t_ = xpool.tile([128, 4096], bf16, name=f"x{b}g{g}",
                                tag="xg")
                x_dmas.append(nc.sync.dma_start(t_[:, :], xp[b, g]))
                xg[b][g] = t_

        for g in range(NG):
            x_dma(g)
        for rg in range(NRG):
            t_ = rpool.tile([128, 4 * TPC], f8, name=f"r{rg}", tag="rr")
            rd = nc.gpsimd.dma_start(t_[:, :], rp[rg])
            if rg == 0:
                add_dep_helper(rd.ins, x_dmas[3].ins,
                               reason="delay r stream behind x groups 0-1")
            rr[rg] = t_

        z_sb = [zpool.tile([128, 32 * 128], bf16, name=f"z{b}", tag="z")
                for b in range(BPC)]
        sq_sb = [sqpool.tile([H, TPC], f32, name=f"sq{b}", tag="sq")
                 for b in range(BPC)]
        # one full PSUM bank per batch: start_tensor_calc zeroes a 2KB
        # region, so interleaved accumulation groups must not share a bank
        cs_ps = [pcs.tile([128, 512], f32, name=f"cs{b}", tag="cs")
                 for b in range(BPC)]

        # --- phase 1: Z = [eKV | eK] per 512-s block; colsum via tiny
        # z-as-weights matmuls accumulating [2H,1] per batch ---
        def ph1_group(g):
            for b in range(BPC):
                for sg in range(2):
                    pkv = pps.tile([128, 512], f32, name="pkv", tag="ps")
                    for si in range(4):
                        sc = g * 8 + sg * 4 + si
                        xoff = (sg * 4 + si) * 128
                        for d in range(4):
                            nc.tensor.matmul(
                                pkv[:, si * 128:(si + 1) * 128],
                                lhsT=xg[b][g][:, d * 1024 + xoff:
                                              d * 1024 + xoff + 128],
                                rhs=wkv_sb[:, d * 128:(d + 1) * 128],
                                start=(d == 0),
                                stop=(not use_bias and d == 3))
                        if use_bias:
                            nc.tensor.matmul(
                                pkv[:, si * 128:(si + 1) * 128],
                                lhsT=ones_sb[:, :], rhs=bkv_sb[:, :],
                                start=False, stop=True)
                    c0 = (g * 8 + sg * 4) * 128
                    pk3 = pkv[:, :].rearrange("p (c k) -> p c k", c=4)
                    zg3 = z_sb[b][:, c0:c0 + 512].rearrange(
                        "p (c k) -> p c k", c=4)
                    nc.scalar.activation(
                        zg3[:, :, H:2 * H], pk3[:, :, 0:H], Exp)
                    nc.vector.tensor_tensor(
                        zg3[:, :, 0:H], pk3[:, :, H:2 * H],
                        zg3[:, :, H:2 * H], mult)
                for si in range(8):
                    sc = g * 8 + si
                    nc.tensor.matmul(
                        cs_ps[b][:, 0:1],
                        lhsT=z_sb[b][:, sc * 128:(sc + 1) * 128],
                        rhs=col64[:, :],
                        start=(sc == 0), stop=(sc == 31),
                        skip_group_check=True)

        def q_block(b):
            for qb in range(4):
                pq = pps.tile([128, 512], f32, name="pq", tag="ps")
                for d in range(4):
                    nc.tensor.matmul(
                        pq[0:H, :],
                        lhsT=wq_sb[:, d * H:(d + 1) * H],
                        rhs=xg[b][qb // 2][:, d * 1024 + (qb % 2) * 512:
                                           d * 1024 + (qb % 2) * 512 + 512],
                        start=(d == 0), stop=(d == 3))
                nc.scalar.activation(
                    sq_sb[b][:, qb * 512:(qb + 1) * 512], pq[0:H, :],
                    Sigmoid, bias=bq_sb[:, :])

        # --- phase 2: nd[p][b] += z_sc^T @ r_sc over all 32 s-chunks ---
        nd = [[None] * BPC for _ in range(NP)]

        def nd_alloc(passes):
            for p in passes:
                for b in range(BPC):
                    nd[p][b] = pnd.tile([128, TP], f32, name=f"nd{p}{b}",
                                        tag="nd")

        def ph2_rg(rg, passes):
            for j in range(4):
                sc = rg * 4 + j
                for b in range(BPC):
                    for p in passes:
                        nc.tensor.matmul(
                            nd[p][b][:, :],
                            lhsT=z_sb[b][:, sc * 128:(sc + 1) * 128],
                            rhs=rr[rg][:, j * TPC + p * TP:
                                       j * TPC + (p + 1) * TP],
                            start=(sc == 0), stop=(sc == 31))

        # --- phase 3 ---
        csv = cpool.tile([128, BPC], f32, name="csv", tag="csv")
        tmps = {}

        def ph3_reads(p, b):
            ncp = tpool.tile([H, TP], f32, name="ncp", tag="ncp")
            nc.scalar.activation(ncp[:, :], nd[p][b][0:H, :], Identity,
                                 bias=csv[0:H, b:b + 1])
            dcp = tpool.tile([H, TP], f32, name="dcp", tag="dcp")
            nc.scalar.activation(dcp[:, :], nd[p][b][H:2 * H, :], Identity,
                                 bias=csv[H:2 * H, b:b + 1])
            rec = tpool.tile([H, TP], f32, name="rec", tag="rec")
            nc.vector.reciprocal_approx_fast(rec[:, :], dcp[:, :])
            tmp = tpool.tile([H, TP], f32, name="tmp", tag="tmp")
            nc.vector.tensor_tensor(tmp[:, :], ncp[:, :], rec[:, :], mult)
            tmps[(p, b)] = tmp

        def ph3_tail(p, b):
            yt = ytpool.tile([H + 1, TP], bf16, name="yt", tag="yt")
            nc.gpsimd.memset(yt[H:H + 1, :], 1.0)
            nc.gpsimd.tensor_tensor(
                yt[0:H, :], tmps[(p, b)][:, :],
                sq_sb[b][:, p * TP:(p + 1) * TP], mult)
            for c in range(4):
                po = pps.tile([128, 512], f32, name="po", tag="ps")
                nc.tensor.matmul(po[:, :], lhsT=yt[:, c * 128:(c + 1) * 128],
                                 rhs=wp_sb[:, :], start=True, stop=True)
                osb = opool.tile([128, 512], bf16, name="osb", tag="osb")
                if c % 2 == 0:
                    nc.scalar.copy(osb[:, :], po[:, :])
                else:
                    nc.vector.tensor_copy(osb[:, :], po[:, :])
                t0 = p * TP + c * 128
                eng = (nc.gpsimd, nc.sync)[c % 2]
                eng.dma_start(out[b, t0:t0 + 128, :], osb[:, :])

        # --- emission schedule (PE order = program order per engine) ---
        ph1_group(0)
        ph1_group(1)
        q_block(0)
        q_block(1)
        ph1_group(2)
        nd_alloc((0, 1))
        ph2_rg(0, (0, 1))
        ph2_rg(1, (0, 1))
        ph1_group(3)
        for b in range(BPC):
            nc.scalar.copy(csv[:, b:b + 1], cs_ps[b][:, 0:1])
        for rg in range(2, NRG):
            ph2_rg(rg, (0, 1))
        for p in (0, 1):
            for b in range(BPC):
                ph3_reads(p, b)
        nd_alloc((2,))
        ph2_rg(0, (2,))
        ph2_rg(1, (2,))
        ph3_tail(0, 0)
        ph3_tail(0, 1)
        ph2_rg(2, (2,))
        ph2_rg(3, (2,))
        ph3_tail(1, 0)
        ph3_tail(1, 1)
        for rg in range(4, NRG):
            ph2_rg(rg, (2,))
        for b in range(BPC):
            ph3_reads(2, b)
        nd_alloc((3,))
        ph2_rg(0, (3,))
        ph2_rg(1, (3,))
        ph3_tail(2, 0)
        ph2_rg(2, (3,))
        ph2_rg(3, (3,))
        ph3_tail(2, 1)
        for rg in range(4, NRG):
            ph2_rg(rg, (3,))
        for b in range(BPC):
            ph3_reads(3, b)
        ph3_tail(3, 0)
        ph3_tail(3, 1)

    nc.compile()
    from concourse.bass_interp import get_hw_module
    nc.m = get_hw_module(nc.m)
    return nc


def _get_module(use_bias):
    key = ("nc", use_bias)
    if key not in _NC_CACHE:
        _NC_CACHE[key] = _build_module(use_bias)
    return _NC_CACHE[key]


def kernel(x, wq, bq, wk, bk, wv, bv, wp, bp, wbias):
    from concourse.bass_utils import run_bass_kernel_spmd

    bf16 = ml_dtypes.bfloat16
    f8 = ml_dtypes.float8_e3m4
    x = np.asarray(x, np.float32)
    wbias = np.asarray(wbias, np.float32)
    wq, wk, wv, wp = (np.asarray(a, np.float32) for a in (wq, wk, wv, wp))
    bq, bk, bv, bp = (np.asarray(a, np.float32) for a in (bq, bk, bv, bp))

    xT_full = np.ascontiguousarray(x.transpose(0, 2, 1)).astype(bf16)
    rq_full = (SCALE * (np.exp(wbias) - 1.0)).T     # [s, t] f32

    wkv_h = np.concatenate([wk.T, wv.T], axis=1).astype(bf16)      # [D, 2H]
    wqT_h = np.ascontiguousarray(wq.T).astype(bf16)                # [D, H]
    wpT_h = np.concatenate(
        [wp.T, np.asarray(bp, np.float32)[None, :]], axis=0).astype(bf16)
    bkv_h = np.concatenate([bk, bv])[None, :].astype(bf16)         # [1, 2H]
    bq_h = np.asarray(bq, np.float32)[:, None].copy()              # [H, 1]
    ones_h = np.ones((1, 128), dtype=bf16)
    use_bias = bool(np.any(bk) or np.any(bv))

    # Per t-group: s-permuted inputs (own t-slice rows first) so the SPMD
    # graph reads Q's x columns at [0:TPC] on every core.
    perm = {}
    for tj in range(TG):
        perm[tj] = np.concatenate([
            np.arange(tj * TPC, (tj + 1) * TPC),
            np.arange(0, tj * TPC),
            np.arange((tj + 1) * TPC, T)])

    rp_tj = {}
    for tj in range(TG):
        rq = rq_full[perm[tj]][:, tj * TPC:(tj + 1) * TPC].astype(f8)
        rp_tj[tj] = np.ascontiguousarray(
            rq.reshape(NRG, 4, 128, TPC).transpose(0, 2, 1, 3)
            .reshape(NRG, 128, 4 * TPC))

    xp_c = {}
    for bi in range(BG):
        for tj in range(TG):
            xt = xT_full[bi * BPC:(bi + 1) * BPC][:, :, perm[tj]]
            xp_c[(bi, tj)] = np.ascontiguousarray(
                xt.reshape(BPC, 4, 128, NG, 1024).transpose(0, 3, 2, 1, 4)
                .reshape(BPC, NG, 128, 4096))

    in_maps = []
    for c in range(N_CORES):
        bi, tj = c // TG, c % TG
        in_maps.append({
            "xp": xp_c[(bi, tj)],
            "rp": rp_tj[tj],
            "wkv": wkv_h, "wqT": wqT_h, "wpT": wpT_h,
            "bkv": bkv_h, "bqv": bq_h, "ones": ones_h,
        })

    nc = _get_module(use_bias)
    res = run_bass_kernel_spmd(nc, in_maps, core_ids=list(range(N_CORES)))

    full = np.empty((B, T, D), dtype=np.float32)
    for c in range(N_CORES):
        bi, tj = c // TG, c % TG
        full[bi * BPC:(bi + 1) * BPC, tj * TPC:(tj + 1) * TPC, :] = \
            res.results[c]["out"].astype(np.float32)
    return full
